# revision 1
# baseline (speedup 1.0000x reference)
"""Causal self-attention (GQA, RoPE, start_pos=0) on 8 Trainium2 cores.

Sharding: tensor-parallel over KV-head groups. Core c owns KV head c and
query heads 4c..4c+3 (w_qkv rows sharded), plus the matching w_proj
columns. Each core computes a full-shape partial output; host sums.

All matmuls run as float32r (full PE rate at moving-dim >= 256,
~tf32-level rounding). Host pre-transposes x -> x^T so QKV projection
produces q^T/k^T/v^T in [head_dim, tokens] layout directly. RoPE is
applied in a de-interleaved head-dim layout (even dims in partitions
0..63, odd in 64..127 — achieved by permuting w_qkv rows on the host) so
the rotation is plain half-tile multiplies. v is transposed back to
[tokens, head_dim] on the PE. Softmax runs without max-subtraction
(scores are O(6) here); the denominator comes from an all-ones matmul
which lands the sum replicated across all 128 partitions.
"""

import sys

for _p in ("/opt/trn_rl_repo", "/root/.axon_site/_ro/trn_rl_repo"):
    if _p not in sys.path:
        sys.path.insert(0, _p)

import numpy as np

B, T, C = 2, 2048, 4096
NT = B * T
N_HEAD, N_KV, HD = 32, 8, 128
N_CORES = 8
QH = N_HEAD // N_KV  # query heads per core
FME = (QH + 2) * HD  # per-core qkv output features: 512 q + 128 k + 128 v
SCL = float(1.0 / np.sqrt(HD))

_cache = {}


def _build():
    import concourse.bacc as bacc
    import concourse.mybir as mybir
    import concourse.tile as tile

    F32R = mybir.dt.float32r
    F32 = mybir.dt.float32
    Exp = mybir.ActivationFunctionType.Exp

    nc = bacc.Bacc("TRN2", target_bir_lowering=False, debug=False,
                   num_devices=N_CORES)

    xT = nc.dram_tensor("xT", [C, NT], F32R, kind="ExternalInput").ap()
    wT = nc.dram_tensor("wT", [C, FME], F32R, kind="ExternalInput").ap()
    wpT = nc.dram_tensor("wpT", [QH * HD, C], F32R, kind="ExternalInput").ap()
    ccT = nc.dram_tensor("ccT", [HD, T], F32R, kind="ExternalInput").ap()
    ssT = nc.dram_tensor("ssT", [HD, T], F32R, kind="ExternalInput").ap()
    msk = nc.dram_tensor("msk", [128, 4 * 512], F32R, kind="ExternalInput").ap()
    ones_d = nc.dram_tensor("ones_d", [128, 128], F32R, kind="ExternalInput").ap()
    id_d = nc.dram_tensor("id_d", [128, 128], F32R, kind="ExternalInput").ap()
    out_d = nc.dram_tensor("out", [NT, C], F32, kind="ExternalOutput").ap()

    TCH = 256                # phase-1 token chunk
    NCH = NT // TCH          # 16
    CT = C // 128            # 32 contraction tiles
    NF = FME // 128          # 6 feature tiles: 0..3 q, 4 k, 5 v

    with tile.TileContext(nc) as tc:
        with tc.tile_pool(name="dram", bufs=1, space="DRAM") as dr, \
             tc.tile_pool(name="tbl", bufs=1) as tbl:
            qT_s = dr.tile([QH * HD, NT], F32R)
            kT_s = dr.tile([HD, NT], F32R)
            v_s = dr.tile([NT, HD], F32R)
            yT_s = dr.tile([QH * HD, NT], F32R)

            cc_sb = tbl.tile([HD, T], F32R)
            ss_sb = tbl.tile([HD, T], F32R)
            nc.sync.dma_start(out=cc_sb, in_=ccT)
            nc.sync.dma_start(out=ss_sb, in_=ssT)

            # ---------------- Phase 1: QKV projection + RoPE + v transpose
            with tc.tile_pool(name="wq", bufs=1) as wq, \
                 tc.tile_pool(name="xp", bufs=2) as xp, \
                 tc.tile_pool(name="ep", bufs=3) as ep, \
                 tc.tile_pool(name="pp", bufs=1, space="PSUM") as pp, \
                 tc.tile_pool(name="tp", bufs=2, space="PSUM") as tp:
                w_sb = wq.tile([128, CT, FME], F32R)
                nc.sync.dma_start(
                    out=w_sb, in_=wT.rearrange("(ct p) f -> p ct f", p=128))
                id_sb = wq.tile([128, 128], F32R)
                nc.sync.dma_start(out=id_sb, in_=id_d)

                for ch in range(NCH):
                    t0 = ch * TCH
                    tb = t0 % T  # batch-relative position for rope tables
                    xc = xp.tile([128, CT, TCH], F32R, tag="xc")
                    nc.sync.dma_start(
                        out=xc,
                        in_=xT.rearrange("(ct p) n -> p ct n", p=128)[:, :, t0:t0 + TCH])
                    for f in range(NF):
                        pf = pp.tile([128, TCH], F32, tag=f"mm{f}")
                        for ct in range(CT):
                            nc.tensor.matmul(
                                pf, w_sb[:, ct, f * 128:(f + 1) * 128],
                                xc[:, ct, :], start=(ct == 0), stop=(ct == CT - 1))
                        if f < 5:  # q heads / k head: rope
                            m1 = ep.tile([128, TCH], F32, tag="m1")
                            m2 = ep.tile([128, TCH], F32, tag="m2")
                            ro = ep.tile([128, TCH], F32R, tag="ro")
                            nc.vector.tensor_mul(m1, pf, cc_sb[:, tb:tb + TCH])
                            nc.vector.tensor_mul(
                                m2[0:64], pf[64:128], ss_sb[0:64, tb:tb + TCH])
                            nc.vector.tensor_mul(
                                m2[64:128], pf[0:64], ss_sb[64:128, tb:tb + TCH])
                            nc.vector.tensor_add(ro, m1, m2)
                            if f < QH:
                                dst = qT_s[f * 128:(f + 1) * 128, t0:t0 + TCH]
                            else:
                                dst = kT_s[:, t0:t0 + TCH]
                            nc.sync.dma_start(out=dst, in_=ro)
                        else:  # v: round to f32r, transpose to [tokens, hd]
                            vc = ep.tile([128, TCH], F32R, tag="vc")
                            nc.vector.tensor_copy(vc, pf)
                            for hf in range(TCH // 128):
                                pt = tp.tile([128, 128], F32R, tag="tr")
                                nc.tensor.transpose(
                                    pt, vc[:, hf * 128:(hf + 1) * 128], id_sb)
                                vo = ep.tile([128, 128], F32R, tag="vo")
                                nc.vector.tensor_copy(vo, pt)
                                r0 = t0 + hf * 128
                                nc.sync.dma_start(
                                    out=v_s[r0:r0 + 128, :], in_=vo)

            # ---------------- Phase 2: attention per (batch, q head)
            with tc.tile_pool(name="kv", bufs=2) as kv, \
                 tc.tile_pool(name="qp", bufs=2) as qp, \
                 tc.tile_pool(name="exb", bufs=6) as exb, \
                 tc.tile_pool(name="wk", bufs=4) as wk, \
                 tc.tile_pool(name="msks", bufs=1) as msks, \
                 tc.tile_pool(name="scp", bufs=3, space="PSUM") as scp, \
                 tc.tile_pool(name="yp", bufs=2, space="PSUM") as yp, \
                 tc.tile_pool(name="smp", bufs=2, space="PSUM") as smp:
                msk_sb = msks.tile([128, 4 * 512], F32R)
                nc.sync.dma_start(out=msk_sb, in_=msk)
                ones_sb = msks.tile([128, 128], F32R)
                nc.sync.dma_start(out=ones_sb, in_=ones_d)

                for b in range(B):
                    g0 = b * T
                    ktb = kv.tile([128, T], F32R, tag="kt")
                    nc.sync.dma_start(out=ktb, in_=kT_s[:, g0:g0 + T])
                    vb = kv.tile([128, T // 128, 128], F32R, tag="vb")
                    nc.sync.dma_start(
                        out=vb,
                        in_=v_s[g0:g0 + T, :].rearrange("(i p) d -> p i d", p=128))
                    for h in range(QH):
                        qtb = qp.tile([128, T], F32R, tag="qt")
                        nc.sync.dma_start(
                            out=qtb, in_=qT_s[h * 128:(h + 1) * 128, g0:g0 + T])
                        for j in range(T // 512):
                            nb = 4 * (j + 1)
                            py_ = yp.tile([128, 512], F32, tag="y")
                            ps_ = smp.tile([128, 512], F32, tag="s")
                            exq = []

                            def drain(k2):
                                nc.tensor.matmul(
                                    ps_, ones_sb, exq[k2],
                                    start=(k2 == 0), stop=(k2 == nb - 1))
                                nc.tensor.matmul(
                                    py_, vb[:, k2, :], exq[k2],
                                    start=(k2 == 0), stop=(k2 == nb - 1))

                            for i in range(nb):
                                sc = scp.tile([128, 512], F32, tag="sc")
                                nc.tensor.matmul(
                                    sc, ktb[:, i * 128:(i + 1) * 128],
                                    qtb[:, j * 512:(j + 1) * 512],
                                    start=True, stop=True)
                                ex = exb.tile([128, 512], F32R, tag="ex")
                                nc.scalar.activation(ex, sc, Exp, scale=SCL)
                                if i >= 4 * j:
                                    d = i - 4 * j
                                    nc.vector.tensor_mul(
                                        ex, ex, msk_sb[:, d * 512:(d + 1) * 512])
                                exq.append(ex)
                                if i >= 2:
                                    drain(i - 2)
                            drain(nb - 2)
                            drain(nb - 1)

                            rec = wk.tile([128, 512], F32, tag="rec")
                            nc.vector.reciprocal(rec, ps_)
                            yst = wk.tile([128, 512], F32R, tag="yst")
                            nc.vector.tensor_mul(yst, py_, rec)
                            c0 = g0 + j * 512
                            nc.sync.dma_start(
                                out=yT_s[h * 128:(h + 1) * 128, c0:c0 + 512],
                                in_=yst)

            # ---------------- Phase 3: output projection (partial out)
            with tc.tile_pool(name="pw", bufs=1) as pw, \
                 tc.tile_pool(name="ost", bufs=3) as ost, \
                 tc.tile_pool(name="op", bufs=4, space="PSUM") as op:
                wp_sb = pw.tile([128, QH, C], F32R)
                nc.sync.dma_start(
                    out=wp_sb, in_=wpT.rearrange("(h p) o -> p h o", p=128))
                y_sb = pw.tile([128, QH, NT], F32R)
                nc.sync.dma_start(
                    out=y_sb, in_=yT_s.rearrange("(h p) n -> p h n", p=128))

                for tt in range(NT // 128):
                    for oc in range(C // 512):
                        po = op.tile([128, 512], F32, tag="o")
                        for h in range(QH):
                            nc.tensor.matmul(
                                po, y_sb[:, h, tt * 128:(tt + 1) * 128],
                                wp_sb[:, h, oc * 512:(oc + 1) * 512],
                                start=(h == 0), stop=(h == QH - 1))
                        ot = ost.tile([128, 512], F32, tag="ot")
                        nc.vector.tensor_copy(ot, po)
                        nc.sync.dma_start(
                            out=out_d[tt * 128:(tt + 1) * 128,
                                      oc * 512:(oc + 1) * 512],
                            in_=ot)

    nc.compile()
    return nc


def _prep_inputs(x, freqs_cos, freqs_sin, w_qkv, w_proj):
    x2 = np.ascontiguousarray(x.reshape(NT, C).T)  # [C, NT]

    deint = np.concatenate([np.arange(0, HD, 2), np.arange(1, HD, 2)])
    cosT = np.ascontiguousarray(freqs_cos.T)  # [64, T]
    sinT = np.ascontiguousarray(freqs_sin.T)
    cc = np.concatenate([cosT, cosT], axis=0).astype(np.float32)
    ss = np.concatenate([-sinT, sinT], axis=0).astype(np.float32)

    # 4 diagonal-block causal masks: mask_d[p, n] = 1 iff p + 128*d <= n
    p = np.arange(128)[:, None]
    n = np.arange(512)[None, :]
    masks = np.concatenate(
        [(p + 128 * d <= n).astype(np.float32) for d in range(4)], axis=1)
    masks = np.ascontiguousarray(masks)

    ones128 = np.ones((128, 128), np.float32)
    eye128 = np.eye(128, dtype=np.float32)

    in_maps = []
    for c in range(N_CORES):
        qrows = w_qkv[c * QH * HD:(c + 1) * QH * HD]  # [512, C]
        qd = qrows.reshape(QH, HD, C)[:, deint, :].reshape(QH * HD, C)
        krows = w_qkv[N_HEAD * HD + c * HD: N_HEAD * HD + (c + 1) * HD]
        kd = krows[deint]
        vrows = w_qkv[(N_HEAD + N_KV) * HD + c * HD:
                      (N_HEAD + N_KV) * HD + (c + 1) * HD]
        wc = np.concatenate([qd, kd, vrows], axis=0)  # [768, C]
        wTc = np.ascontiguousarray(wc.T)  # [C, 768]
        wpTc = np.ascontiguousarray(
            w_proj[:, c * QH * HD:(c + 1) * QH * HD].T)  # [512, C]
        in_maps.append({
            "xT": x2, "wT": wTc, "wpT": wpTc, "ccT": cc, "ssT": ss,
            "msk": masks, "ones_d": ones128, "id_d": eye128,
        })
    return in_maps


def kernel(x, freqs_cos, freqs_sin, w_qkv, w_proj, cache_k, cache_v,
           start_pos, _want_results=False, _trace=False):
    from concourse import bass_utils

    assert int(start_pos) == 0
    x = np.asarray(x, dtype=np.float32)
    freqs_cos = np.asarray(freqs_cos, dtype=np.float32)
    freqs_sin = np.asarray(freqs_sin, dtype=np.float32)
    w_qkv = np.asarray(w_qkv, dtype=np.float32)
    w_proj = np.asarray(w_proj, dtype=np.float32)

    if "nc" not in _cache:
        _cache["nc"] = _build()
    nc = _cache["nc"]

    in_maps = _prep_inputs(x, freqs_cos, freqs_sin, w_qkv, w_proj)
    res = bass_utils.run_bass_kernel_spmd(
        nc, in_maps, core_ids=list(range(N_CORES)), trace=_trace)

    acc = res.results[0]["out"].astype(np.float32)
    for c in range(1, N_CORES):
        acc = acc + res.results[c]["out"]
    out = acc.reshape(B, T, C)
    if _want_results:
        return out, res
    return out


# revision 3
# speedup vs baseline: 26523.4035x; 26523.4035x over previous
"""Causal self-attention (GQA, RoPE, start_pos=0) on 8 Trainium2 cores.

Sharding: tensor-parallel over KV-head groups. Core c owns KV head c and
query heads 4c..4c+3 (w_qkv rows sharded), plus the matching w_proj
columns. Each core computes a full-shape partial output; host sums.

All matmuls run as float32r (full PE rate at moving-dim >= 256,
~tf32-level rounding). Host pre-transposes x -> x^T so QKV projection
produces q^T/k^T/v^T in [head_dim, tokens] layout directly. RoPE is
applied in a de-interleaved head-dim layout (even dims in partitions
0..63, odd in 64..127 — achieved by permuting w_qkv rows on the host) so
the rotation is plain half-tile multiplies. v is transposed back to
[tokens, head_dim] on the PE. Softmax runs without max-subtraction
(scores are O(6) here); the denominator comes from an all-ones matmul
which lands the sum replicated across all 128 partitions.
"""

import sys

for _p in ("/opt/trn_rl_repo", "/root/.axon_site/_ro/trn_rl_repo"):
    if _p not in sys.path:
        sys.path.insert(0, _p)

import numpy as np

B, T, C = 2, 2048, 4096
NT = B * T
N_HEAD, N_KV, HD = 32, 8, 128
N_CORES = 8
QH = N_HEAD // N_KV  # query heads per core
FME = (QH + 2) * HD  # per-core qkv output features: 512 q + 128 k + 128 v
SCL = float(1.0 / np.sqrt(HD))

_cache = {}


def _build():
    import concourse.bacc as bacc
    import concourse.mybir as mybir
    import concourse.tile as tile

    F32R = mybir.dt.float32r
    F32 = mybir.dt.float32
    Exp = mybir.ActivationFunctionType.Exp

    nc = bacc.Bacc("TRN2", target_bir_lowering=False, debug=False,
                   num_devices=N_CORES)

    xT = nc.dram_tensor("xT", [C, NT], F32R, kind="ExternalInput").ap()
    wT = nc.dram_tensor("wT", [C, FME], F32R, kind="ExternalInput").ap()
    wpT = nc.dram_tensor("wpT", [QH * HD, C], F32R, kind="ExternalInput").ap()
    ccT = nc.dram_tensor("ccT", [HD, T], F32R, kind="ExternalInput").ap()
    ssT = nc.dram_tensor("ssT", [HD, T], F32R, kind="ExternalInput").ap()
    msk = nc.dram_tensor("msk", [128, 4 * 512], F32R, kind="ExternalInput").ap()
    ones_d = nc.dram_tensor("ones_d", [128, 128], F32R, kind="ExternalInput").ap()
    id_d = nc.dram_tensor("id_d", [128, 128], F32R, kind="ExternalInput").ap()
    out_d = nc.dram_tensor("out", [NT, C], F32, kind="ExternalOutput").ap()

    TCH = 256                # phase-1 token chunk
    NCH = NT // TCH          # 16
    CT = C // 128            # 32 contraction tiles
    NF = FME // 128          # 6 feature tiles: 0..3 q, 4 k, 5 v

    with tile.TileContext(nc) as tc:
        with tc.tile_pool(name="dram", bufs=1, space="DRAM") as dr, \
             tc.tile_pool(name="tbl", bufs=1) as tbl:
            qT_s = dr.tile([QH * HD, NT], F32R)
            kT_s = dr.tile([HD, NT], F32R)
            v_s = dr.tile([NT, HD], F32R)
            yT_s = dr.tile([QH * HD, NT], F32R)

            cc_sb = tbl.tile([HD, T], F32R)
            ss_sb = tbl.tile([HD, T], F32R)
            nc.sync.dma_start(out=cc_sb, in_=ccT)
            nc.sync.dma_start(out=ss_sb, in_=ssT)

            # ---------------- Phase 1: QKV projection + RoPE + v transpose
            with tc.tile_pool(name="wq", bufs=1) as wq, \
                 tc.tile_pool(name="xp", bufs=2) as xp, \
                 tc.tile_pool(name="ep", bufs=3) as ep, \
                 tc.tile_pool(name="pp", bufs=1, space="PSUM") as pp, \
                 tc.tile_pool(name="tp", bufs=2, space="PSUM") as tp:
                w_sb = wq.tile([128, CT, FME], F32R)
                nc.sync.dma_start(
                    out=w_sb, in_=wT.rearrange("(ct p) f -> p ct f", p=128))
                id_sb = wq.tile([128, 128], F32R)
                nc.sync.dma_start(out=id_sb, in_=id_d)

                for ch in range(NCH):
                    t0 = ch * TCH
                    tb = t0 % T  # batch-relative position for rope tables
                    xc = xp.tile([128, CT, TCH], F32R, tag="xc")
                    nc.sync.dma_start(
                        out=xc,
                        in_=xT.rearrange("(ct p) n -> p ct n", p=128)[:, :, t0:t0 + TCH])
                    for f in range(NF):
                        pf = pp.tile([128, TCH], F32, tag=f"mm{f}")
                        for ct in range(CT):
                            nc.tensor.matmul(
                                pf, w_sb[:, ct, f * 128:(f + 1) * 128],
                                xc[:, ct, :], start=(ct == 0), stop=(ct == CT - 1))
                        if f < 5:  # q heads / k head: rope
                            m1 = ep.tile([128, TCH], F32, tag="m1")
                            m2 = ep.tile([128, TCH], F32, tag="m2")
                            ro = ep.tile([128, TCH], F32R, tag="ro")
                            nc.vector.tensor_mul(m1, pf, cc_sb[:, tb:tb + TCH])
                            nc.vector.tensor_mul(
                                m2[0:64], pf[64:128], ss_sb[0:64, tb:tb + TCH])
                            nc.vector.tensor_mul(
                                m2[64:128], pf[0:64], ss_sb[64:128, tb:tb + TCH])
                            nc.vector.tensor_add(ro, m1, m2)
                            if f < QH:
                                dst = qT_s[f * 128:(f + 1) * 128, t0:t0 + TCH]
                            else:
                                dst = kT_s[:, t0:t0 + TCH]
                            nc.sync.dma_start(out=dst, in_=ro)
                        else:  # v: round to f32r, transpose to [tokens, hd]
                            vc = ep.tile([128, TCH], F32R, tag="vc")
                            nc.vector.tensor_copy(vc, pf)
                            for hf in range(TCH // 128):
                                pt = tp.tile([128, 128], F32R, tag="tr")
                                nc.tensor.transpose(
                                    pt, vc[:, hf * 128:(hf + 1) * 128], id_sb)
                                vo = ep.tile([128, 128], F32R, tag="vo")
                                nc.vector.tensor_copy(vo, pt)
                                r0 = t0 + hf * 128
                                nc.sync.dma_start(
                                    out=v_s[r0:r0 + 128, :], in_=vo)

            # ---------------- Phase 2: attention per (batch, q head)
            with tc.tile_pool(name="kv", bufs=2) as kv, \
                 tc.tile_pool(name="qp", bufs=2) as qp, \
                 tc.tile_pool(name="exb", bufs=6) as exb, \
                 tc.tile_pool(name="wk", bufs=4) as wk, \
                 tc.tile_pool(name="msks", bufs=1) as msks, \
                 tc.tile_pool(name="scp", bufs=3, space="PSUM") as scp, \
                 tc.tile_pool(name="yp", bufs=2, space="PSUM") as yp, \
                 tc.tile_pool(name="smp", bufs=2, space="PSUM") as smp:
                msk_sb = msks.tile([128, 4 * 512], F32R)
                nc.sync.dma_start(out=msk_sb, in_=msk)
                ones_sb = msks.tile([128, 128], F32R)
                nc.sync.dma_start(out=ones_sb, in_=ones_d)

                for b in range(B):
                    g0 = b * T
                    ktb = kv.tile([128, T], F32R, tag="kt")
                    nc.sync.dma_start(out=ktb, in_=kT_s[:, g0:g0 + T])
                    vb = kv.tile([128, T // 128, 128], F32R, tag="vb")
                    nc.sync.dma_start(
                        out=vb,
                        in_=v_s[g0:g0 + T, :].rearrange("(i p) d -> p i d", p=128))
                    for h in range(QH):
                        qtb = qp.tile([128, T], F32R, tag="qt")
                        nc.sync.dma_start(
                            out=qtb, in_=qT_s[h * 128:(h + 1) * 128, g0:g0 + T])
                        for j in range(T // 512):
                            nb = 4 * (j + 1)
                            py_ = yp.tile([128, 512], F32, tag="y")
                            ps_ = smp.tile([128, 512], F32, tag="s")
                            exq = []

                            def drain(k2):
                                nc.tensor.matmul(
                                    ps_, ones_sb, exq[k2],
                                    start=(k2 == 0), stop=(k2 == nb - 1))
                                nc.tensor.matmul(
                                    py_, vb[:, k2, :], exq[k2],
                                    start=(k2 == 0), stop=(k2 == nb - 1))

                            for i in range(nb):
                                sc = scp.tile([128, 512], F32, tag="sc")
                                nc.tensor.matmul(
                                    sc, ktb[:, i * 128:(i + 1) * 128],
                                    qtb[:, j * 512:(j + 1) * 512],
                                    start=True, stop=True)
                                ex = exb.tile([128, 512], F32R, tag="ex")
                                nc.scalar.activation(ex, sc, Exp, scale=SCL)
                                if i >= 4 * j:
                                    d = i - 4 * j
                                    nc.vector.tensor_mul(
                                        ex, ex, msk_sb[:, d * 512:(d + 1) * 512])
                                exq.append(ex)
                                if i >= 2:
                                    drain(i - 2)
                            drain(nb - 2)
                            drain(nb - 1)

                            rec = wk.tile([128, 512], F32, tag="rec")
                            nc.vector.reciprocal(rec, ps_)
                            yst = wk.tile([128, 512], F32R, tag="yst")
                            nc.vector.tensor_mul(yst, py_, rec)
                            c0 = g0 + j * 512
                            nc.sync.dma_start(
                                out=yT_s[h * 128:(h + 1) * 128, c0:c0 + 512],
                                in_=yst)

            # ---------------- Phase 3: output projection (partial out)
            with tc.tile_pool(name="pw", bufs=1) as pw, \
                 tc.tile_pool(name="ost", bufs=3) as ost, \
                 tc.tile_pool(name="op", bufs=4, space="PSUM") as op:
                wp_sb = pw.tile([128, QH, C], F32R)
                nc.sync.dma_start(
                    out=wp_sb, in_=wpT.rearrange("(h p) o -> p h o", p=128))
                y_sb = pw.tile([128, QH, NT], F32R)
                nc.sync.dma_start(
                    out=y_sb, in_=yT_s.rearrange("(h p) n -> p h n", p=128))

                for tt in range(NT // 128):
                    for oc in range(C // 512):
                        po = op.tile([128, 512], F32, tag="o")
                        for h in range(QH):
                            nc.tensor.matmul(
                                po, y_sb[:, h, tt * 128:(tt + 1) * 128],
                                wp_sb[:, h, oc * 512:(oc + 1) * 512],
                                start=(h == 0), stop=(h == QH - 1))
                        ot = ost.tile([128, 512], F32, tag="ot")
                        nc.vector.tensor_copy(ot, po)
                        nc.sync.dma_start(
                            out=out_d[tt * 128:(tt + 1) * 128,
                                      oc * 512:(oc + 1) * 512],
                            in_=ot)

    nc.compile()
    return nc


def _prep_inputs(x, freqs_cos, freqs_sin, w_qkv, w_proj):
    x2 = np.ascontiguousarray(x.reshape(NT, C).T)  # [C, NT]

    deint = np.concatenate([np.arange(0, HD, 2), np.arange(1, HD, 2)])
    cosT = np.ascontiguousarray(freqs_cos.T)  # [64, T]
    sinT = np.ascontiguousarray(freqs_sin.T)
    cc = np.concatenate([cosT, cosT], axis=0).astype(np.float32)
    ss = np.concatenate([-sinT, sinT], axis=0).astype(np.float32)

    # 4 diagonal-block causal masks: mask_d[p, n] = 1 iff p + 128*d <= n
    p = np.arange(128)[:, None]
    n = np.arange(512)[None, :]
    masks = np.concatenate(
        [(p + 128 * d <= n).astype(np.float32) for d in range(4)], axis=1)
    masks = np.ascontiguousarray(masks)

    ones128 = np.ones((128, 128), np.float32)
    eye128 = np.eye(128, dtype=np.float32)

    in_maps = []
    for c in range(N_CORES):
        qrows = w_qkv[c * QH * HD:(c + 1) * QH * HD]  # [512, C]
        qd = qrows.reshape(QH, HD, C)[:, deint, :].reshape(QH * HD, C)
        krows = w_qkv[N_HEAD * HD + c * HD: N_HEAD * HD + (c + 1) * HD]
        kd = krows[deint]
        vrows = w_qkv[(N_HEAD + N_KV) * HD + c * HD:
                      (N_HEAD + N_KV) * HD + (c + 1) * HD]
        wc = np.concatenate([qd, kd, vrows], axis=0)  # [768, C]
        wTc = np.ascontiguousarray(wc.T)  # [C, 768]
        wpTc = np.ascontiguousarray(
            w_proj[:, c * QH * HD:(c + 1) * QH * HD].T)  # [512, C]
        in_maps.append({
            "xT": x2, "wT": wTc, "wpT": wpTc, "ccT": cc, "ssT": ss,
            "msk": masks, "ones_d": ones128, "id_d": eye128,
        })
    return in_maps


def kernel(x, freqs_cos, freqs_sin, w_qkv, w_proj, cache_k, cache_v,
           start_pos, _want_results=False, _trace=False, _tmpdir=None):
    from concourse import bass_utils

    assert int(start_pos) == 0
    x = np.asarray(x, dtype=np.float32)
    freqs_cos = np.asarray(freqs_cos, dtype=np.float32)
    freqs_sin = np.asarray(freqs_sin, dtype=np.float32)
    w_qkv = np.asarray(w_qkv, dtype=np.float32)
    w_proj = np.asarray(w_proj, dtype=np.float32)

    if "nc" not in _cache:
        _cache["nc"] = _build()
    nc = _cache["nc"]

    in_maps = _prep_inputs(x, freqs_cos, freqs_sin, w_qkv, w_proj)
    res = bass_utils.run_bass_kernel_spmd(
        nc, in_maps, core_ids=list(range(N_CORES)), trace=_trace,
        tmpdir=_tmpdir)

    acc = res.results[0]["out"].astype(np.float32)
    for c in range(1, N_CORES):
        acc = acc + res.results[c]["out"]
    out = acc.reshape(B, T, C)
    if _want_results:
        return out, res
    return out


# revision 5
# speedup vs baseline: 29746.6026x; 1.1215x over previous
"""Causal self-attention (GQA, RoPE, start_pos=0) on 8 Trainium2 cores.

Sharding: tensor-parallel over KV-head groups. Core c owns KV head c and
query heads 4c..4c+3 (w_qkv rows sharded), plus the matching w_proj
columns. Each core computes a full-shape partial output; host sums.

All matmuls run as float32r (full PE rate at moving-dim >= 256,
~tf32-level rounding). Host pre-transposes x -> x^T so QKV projection
produces q^T/k^T/v^T in [head_dim, tokens] layout directly. RoPE is
applied in a de-interleaved head-dim layout (even dims in partitions
0..63, odd in 64..127 — achieved by permuting w_qkv rows on the host) so
the rotation is plain half-tile multiplies. v is transposed back to
[tokens, head_dim] on the PE. Softmax runs without max-subtraction
(scores are O(6) here); the denominator comes from an all-ones matmul
which lands the sum replicated across all 128 partitions; attention
sum/AV matmuls are pipelined 3 blocks behind the score matmuls so the
PE never waits on the ACT-engine exp.
"""

import sys

for _p in ("/opt/trn_rl_repo", "/root/.axon_site/_ro/trn_rl_repo"):
    if _p not in sys.path:
        sys.path.insert(0, _p)

import numpy as np

B, T, C = 2, 2048, 4096
NT = B * T
N_HEAD, N_KV, HD = 32, 8, 128
N_CORES = 8
QH = N_HEAD // N_KV  # query heads per core
FME = (QH + 2) * HD  # per-core qkv output features: 512 q + 128 k + 128 v
SCL = float(1.0 / np.sqrt(HD))

_cache = {}


def _build():
    import concourse.bacc as bacc
    import concourse.mybir as mybir
    import concourse.tile as tile

    F32R = mybir.dt.float32r
    F32 = mybir.dt.float32
    Exp = mybir.ActivationFunctionType.Exp

    nc = bacc.Bacc("TRN2", target_bir_lowering=False, debug=False,
                   num_devices=N_CORES)

    xT = nc.dram_tensor("xT", [C, NT], F32R, kind="ExternalInput").ap()
    wT = nc.dram_tensor("wT", [C, FME], F32R, kind="ExternalInput").ap()
    wpT = nc.dram_tensor("wpT", [QH * HD, C], F32R, kind="ExternalInput").ap()
    ccT = nc.dram_tensor("ccT", [HD, T], F32R, kind="ExternalInput").ap()
    ssT = nc.dram_tensor("ssT", [HD, T], F32R, kind="ExternalInput").ap()
    msk = nc.dram_tensor("msk", [128, 4 * 512], F32R, kind="ExternalInput").ap()
    ones_d = nc.dram_tensor("ones_d", [128, 128], F32R, kind="ExternalInput").ap()
    id_d = nc.dram_tensor("id_d", [128, 128], F32R, kind="ExternalInput").ap()
    out_d = nc.dram_tensor("out", [NT, C], F32, kind="ExternalOutput").ap()

    TCH = 512                # phase-1 token chunk
    NCH = NT // TCH          # 8
    CT = C // 128            # 32 contraction tiles
    HCT = CT // 2            # 16 per half
    NF = FME // 128          # 6 feature tiles: 0..3 q, 4 k, 5 v

    xTr = xT.rearrange("(ct p) n -> p ct n", p=128)
    wTr = wT.rearrange("(ct p) f -> p ct f", p=128)

    with tile.TileContext(nc) as tc:
        with tc.tile_pool(name="dram", bufs=1, space="DRAM") as dr:
            qT_s = dr.tile([QH * HD, NT], F32R)
            kT_s = dr.tile([HD, NT], F32R)
            v_s = dr.tile([NT, HD], F32R)

            # ---------------- Phase 1: QKV projection + RoPE + v transpose
            with tc.tile_pool(name="wq", bufs=1) as wq, \
                 tc.tile_pool(name="xp", bufs=2) as xp, \
                 tc.tile_pool(name="ep", bufs=3) as ep, \
                 tc.tile_pool(name="pp", bufs=1, space="PSUM") as pp, \
                 tc.tile_pool(name="tp", bufs=2, space="PSUM") as tp:
                w_fs = []
                for f in range(NF):
                    w_f = wq.tile([128, CT, 128], F32R, tag=f"w{f}")
                    nc.sync.dma_start(
                        out=w_f, in_=wTr[:, :, f * 128:(f + 1) * 128])
                    w_fs.append(w_f)
                id_sb = wq.tile([128, 128], F32R)
                nc.sync.dma_start(out=id_sb, in_=id_d)
                cc_sb = wq.tile([HD, T], F32R)
                ss_sb = wq.tile([HD, T], F32R)
                nc.sync.dma_start(out=cc_sb, in_=ccT)
                nc.sync.dma_start(out=ss_sb, in_=ssT)

                for ch in range(NCH):
                    t0 = ch * TCH
                    tb = t0 % T  # batch-relative position for rope tables
                    pfs = [pp.tile([128, TCH], F32, tag=f"mm{f}", name=f"pf{f}")
                           for f in range(NF)]
                    for half in range(2):
                        xc = xp.tile([128, HCT, TCH], F32R, tag="xc")
                        nc.sync.dma_start(
                            out=xc,
                            in_=xTr[:, half * HCT:(half + 1) * HCT, t0:t0 + TCH])
                        for f in range(NF):
                            for ct in range(HCT):
                                nc.tensor.matmul(
                                    pfs[f],
                                    w_fs[f][:, half * HCT + ct, :],
                                    xc[:, ct, :],
                                    start=(half == 0 and ct == 0),
                                    stop=(half == 1 and ct == HCT - 1))
                    for f in range(NF):
                        pf = pfs[f]
                        if f < 5:  # q heads / k head: rope
                            m1 = ep.tile([128, TCH], F32, tag="m1")
                            m2 = ep.tile([128, TCH], F32, tag="m2")
                            ro = ep.tile([128, TCH], F32R, tag="ro")
                            nc.vector.tensor_mul(m1, pf, cc_sb[:, tb:tb + TCH])
                            nc.vector.tensor_mul(
                                m2[0:64], pf[64:128], ss_sb[0:64, tb:tb + TCH])
                            nc.vector.tensor_mul(
                                m2[64:128], pf[0:64], ss_sb[64:128, tb:tb + TCH])
                            nc.vector.tensor_add(ro, m1, m2)
                            if f < QH:
                                dst = qT_s[f * 128:(f + 1) * 128, t0:t0 + TCH]
                            else:
                                dst = kT_s[:, t0:t0 + TCH]
                            nc.sync.dma_start(out=dst, in_=ro)
                        else:  # v: round to f32r, transpose to [tokens, hd]
                            vc = ep.tile([128, TCH], F32R, tag="vc")
                            nc.vector.tensor_copy(vc, pf)
                            for hf in range(TCH // 128):
                                pt = tp.tile([128, 128], F32R, tag="tr")
                                nc.tensor.transpose(
                                    pt, vc[:, hf * 128:(hf + 1) * 128], id_sb)
                                vo = ep.tile([128, 128], F32R, tag="vo")
                                nc.vector.tensor_copy(vo, pt)
                                r0 = t0 + hf * 128
                                nc.sync.dma_start(
                                    out=v_s[r0:r0 + 128, :], in_=vo)

            # ---------------- Phases 2+3 shared: yT resident + w_proj
            with tc.tile_pool(name="p23", bufs=1) as p23:
                yT_sb = p23.tile([128, QH, NT], F32R)
                wp_sb = p23.tile([128, QH, C], F32R)

                # ---------------- Phase 2: attention per (batch, q head)
                with tc.tile_pool(name="kv", bufs=1) as kv, \
                     tc.tile_pool(name="qp", bufs=2) as qp, \
                     tc.tile_pool(name="exb", bufs=6) as exb, \
                     tc.tile_pool(name="wk", bufs=4) as wk, \
                     tc.tile_pool(name="msks", bufs=1) as msks, \
                     tc.tile_pool(name="scp", bufs=3, space="PSUM") as scp, \
                     tc.tile_pool(name="yp", bufs=2, space="PSUM") as yp, \
                     tc.tile_pool(name="smp", bufs=2, space="PSUM") as smp:
                    msk_sb = msks.tile([128, 4 * 512], F32R)
                    nc.sync.dma_start(out=msk_sb, in_=msk)
                    ones_sb = msks.tile([128, 128], F32R)
                    nc.sync.dma_start(out=ones_sb, in_=ones_d)

                    LAG = 3
                    pend = []  # (state, i) awaiting sum/AV matmuls

                    def flush_one():
                        st, i = pend.pop(0)
                        nc.tensor.matmul(
                            st["ps"], ones_sb, st["ex"][i],
                            start=(i == 0), stop=(i == st["nb"] - 1))
                        nc.tensor.matmul(
                            st["py"], st["vb"][:, i, :], st["ex"][i],
                            start=(i == 0), stop=(i == st["nb"] - 1))
                        if i == st["nb"] - 1:
                            rec = wk.tile([128, 512], F32, tag="rec")
                            nc.vector.reciprocal_approx_fast(rec, st["ps"])
                            nc.vector.tensor_mul(
                                yT_sb[:, st["h"], st["c0"]:st["c0"] + 512],
                                st["py"], rec)

                    first = True
                    for b in range(B):
                        g0 = b * T
                        ktb = kv.tile([128, T], F32R, tag="kt")
                        nc.sync.dma_start(out=ktb, in_=kT_s[:, g0:g0 + T])
                        vb = kv.tile([128, T // 128, 128], F32R, tag="vb")
                        nc.sync.dma_start(
                            out=vb,
                            in_=v_s[g0:g0 + T, :].rearrange(
                                "(i p) d -> p i d", p=128))
                        for h in range(QH):
                            qtb = qp.tile([128, T], F32R, tag="qt")
                            nc.sync.dma_start(
                                out=qtb,
                                in_=qT_s[h * 128:(h + 1) * 128, g0:g0 + T])
                            if first:
                                # prefetch phase-3 weights during phase 2
                                nc.sync.dma_start(
                                    out=wp_sb,
                                    in_=wpT.rearrange("(h p) o -> p h o", p=128))
                                first = False
                            for j in range(T // 512):
                                nb = 4 * (j + 1)
                                st = {
                                    "nb": nb, "h": h, "c0": g0 + j * 512,
                                    "vb": vb, "ex": [],
                                    "py": yp.tile([128, 512], F32, tag="y",
                                                   name="py"),
                                    "ps": smp.tile([128, 512], F32, tag="s",
                                                   name="ps"),
                                }
                                for i in range(nb):
                                    sc = scp.tile([128, 512], F32, tag="sc")
                                    nc.tensor.matmul(
                                        sc, ktb[:, i * 128:(i + 1) * 128],
                                        qtb[:, j * 512:(j + 1) * 512],
                                        start=True, stop=True)
                                    ex = exb.tile([128, 512], F32R, tag="ex")
                                    nc.scalar.activation(ex, sc, Exp, scale=SCL)
                                    if i >= 4 * j:
                                        d = i - 4 * j
                                        nc.vector.tensor_mul(
                                            ex, ex,
                                            msk_sb[:, d * 512:(d + 1) * 512])
                                    st["ex"].append(ex)
                                    pend.append((st, i))
                                    if len(pend) > LAG:
                                        flush_one()
                    while pend:
                        flush_one()

                # ---------------- Phase 3: output projection (partial out)
                with tc.tile_pool(name="ost", bufs=3) as ost, \
                     tc.tile_pool(name="op", bufs=4, space="PSUM") as op:
                    for tt in range(NT // 128):
                        for oc in range(C // 512):
                            po = op.tile([128, 512], F32, tag="o")
                            for h in range(QH):
                                nc.tensor.matmul(
                                    po, yT_sb[:, h, tt * 128:(tt + 1) * 128],
                                    wp_sb[:, h, oc * 512:(oc + 1) * 512],
                                    start=(h == 0), stop=(h == QH - 1))
                            ot = ost.tile([128, 512], F32, tag="ot")
                            nc.vector.tensor_copy(ot, po)
                            nc.sync.dma_start(
                                out=out_d[tt * 128:(tt + 1) * 128,
                                          oc * 512:(oc + 1) * 512],
                                in_=ot)

    nc.compile()
    return nc


def _prep_inputs(x, freqs_cos, freqs_sin, w_qkv, w_proj):
    x2 = np.ascontiguousarray(x.reshape(NT, C).T)  # [C, NT]

    deint = np.concatenate([np.arange(0, HD, 2), np.arange(1, HD, 2)])
    cosT = np.ascontiguousarray(freqs_cos.T)  # [64, T]
    sinT = np.ascontiguousarray(freqs_sin.T)
    cc = np.concatenate([cosT, cosT], axis=0).astype(np.float32)
    ss = np.concatenate([-sinT, sinT], axis=0).astype(np.float32)

    # 4 diagonal-block causal masks: mask_d[p, n] = 1 iff p + 128*d <= n
    p = np.arange(128)[:, None]
    n = np.arange(512)[None, :]
    masks = np.concatenate(
        [(p + 128 * d <= n).astype(np.float32) for d in range(4)], axis=1)
    masks = np.ascontiguousarray(masks)

    ones128 = np.ones((128, 128), np.float32)
    eye128 = np.eye(128, dtype=np.float32)

    in_maps = []
    for c in range(N_CORES):
        qrows = w_qkv[c * QH * HD:(c + 1) * QH * HD]  # [512, C]
        qd = qrows.reshape(QH, HD, C)[:, deint, :].reshape(QH * HD, C)
        krows = w_qkv[N_HEAD * HD + c * HD: N_HEAD * HD + (c + 1) * HD]
        kd = krows[deint]
        vrows = w_qkv[(N_HEAD + N_KV) * HD + c * HD:
                      (N_HEAD + N_KV) * HD + (c + 1) * HD]
        wc = np.concatenate([qd, kd, vrows], axis=0)  # [768, C]
        wTc = np.ascontiguousarray(wc.T)  # [C, 768]
        wpTc = np.ascontiguousarray(
            w_proj[:, c * QH * HD:(c + 1) * QH * HD].T)  # [512, C]
        in_maps.append({
            "xT": x2, "wT": wTc, "wpT": wpTc, "ccT": cc, "ssT": ss,
            "msk": masks, "ones_d": ones128, "id_d": eye128,
        })
    return in_maps


def kernel(x, freqs_cos, freqs_sin, w_qkv, w_proj, cache_k, cache_v,
           start_pos, _want_results=False, _trace=False, _tmpdir=None):
    from concourse import bass_utils

    assert int(start_pos) == 0
    x = np.asarray(x, dtype=np.float32)
    freqs_cos = np.asarray(freqs_cos, dtype=np.float32)
    freqs_sin = np.asarray(freqs_sin, dtype=np.float32)
    w_qkv = np.asarray(w_qkv, dtype=np.float32)
    w_proj = np.asarray(w_proj, dtype=np.float32)

    if "nc" not in _cache:
        _cache["nc"] = _build()
    nc = _cache["nc"]

    in_maps = _prep_inputs(x, freqs_cos, freqs_sin, w_qkv, w_proj)
    res = bass_utils.run_bass_kernel_spmd(
        nc, in_maps, core_ids=list(range(N_CORES)), trace=_trace,
        tmpdir=_tmpdir)

    acc = res.results[0]["out"].astype(np.float32)
    for c in range(1, N_CORES):
        acc = acc + res.results[c]["out"]
    out = acc.reshape(B, T, C)
    if _want_results:
        return out, res
    return out
